# revision 13
# baseline (speedup 1.0000x reference)
"""EGNN message-passing layer (CDVAE) on 8 Trainium2 NeuronCores.

Strategy (edge-parallel, destination-sharded — no collectives):
  * Edges are sharded by destination-node range: core c owns all edges whose
    dst lies in [c*NLOC, (c+1)*NLOC).  The segment-sum therefore lands
    entirely in the core's own node slice and no cross-core reduction is
    needed; the host concatenates per-core output slices.
  * Within a core, edges are grouped by 128-node destination window (for the
    PSUM-accumulated one-hot matmul scatter) and by source-table half (the
    dma_gather index dtype is int16, so the gather table is split into two
    halves of < 32768 rows).
  * Node features are gathered with the transposed bf16 dma_gather, which
    lands features on partitions ([feat, edge] layout) so the whole edge MLP
    runs weight-stationary with no per-tile transposes.  Per-edge coords are
    host-gathered (24B/edge) and streamed; the heavyweight feature gather
    stays on device.
  * Scatter: per 128-edge tile, a one-hot matrix onehot[e, n] (built with
    iota + tensor_scalar is_equal from precomputed window-local dst indices)
    multiplies [msgT | weighted_coord_diff] in one matmul, accumulating
    [n, 128+3] in a per-window PSUM tile.  The node MLP + residual and the
    coord residual run per window as the window closes.
"""

import os

import numpy as np
import ml_dtypes

import concourse.bass as bass
import concourse.tile as tile
from concourse import bacc, mybir
from concourse.bass_utils import run_bass_kernel_spmd
from concourse.masks import make_identity

F32 = mybir.dt.float32
BF16 = mybir.dt.bfloat16
I16 = mybir.dt.int16
I32 = mybir.dt.int32


def _patch_sim_silu():
    """The instruction simulator (also used by Tile's scheduler) lacks Silu;
    hardware supports it via the silu_and_others activation table."""
    import concourse.bass_interp as bi

    if getattr(bi.InstructionExecutor, "_silu_patched", False):
        return
    _orig = bi.InstructionExecutor.visit_InstActivation

    def _patched(self, instruction, *, reg_snapshot=None):
        f = instruction.func
        if f == mybir.ActivationFunctionType.Silu:
            instruction.func = mybir.ActivationFunctionType.Identity
            try:
                res = _orig(self, instruction, reg_snapshot=reg_snapshot)
            finally:
                instruction.func = f
            v = self.view_ap(instruction.outs[0], bi.Direction.WRITE,
                             instruction, reg_snapshot=reg_snapshot)
            x = np.asarray(v, np.float64)
            v[...] = (x / (1.0 + np.exp(-x))).astype(
                v.dtype if hasattr(v, "dtype") else np.float32)
            return res
        return _orig(self, instruction, reg_snapshot=reg_snapshot)

    bi.InstructionExecutor.visit_InstActivation = _patched
    bi.InstructionExecutor._silu_patched = True


_patch_sim_silu()

N_CORES = 8
P = 128          # partition / tile granularity
WIN = 128        # nodes per scatter window
MAX_RUN_TILES = 16   # max 128-edge tiles per dma_gather call
CHUNK_TILES = 4      # MLP batch = up to 4 tiles (512 edges)
EPS = 1e-8

LAST_RESULTS = None  # BassKernelResults of the most recent kernel() call


def _bf(x):
    return np.ascontiguousarray(x.astype(ml_dtypes.bfloat16))


def _f32(x):
    return np.ascontiguousarray(x.astype(np.float32))


# ---------------------------------------------------------------------------
# Host-side sharding / layout prep
# ---------------------------------------------------------------------------

def _prepare(inputs):
    nf = np.asarray(inputs["node_feat"], np.float32)
    ea = np.asarray(inputs["edge_attr"], np.float32)
    co = np.asarray(inputs["coords"], np.float32)
    ei = np.asarray(inputs["edge_index"], np.int32)
    src, dst = ei[0], ei[1]

    n_nodes, nd = nf.shape
    n_edges, ed = ea.shape
    assert nd == 128 and ed == 64
    assert n_nodes % N_CORES == 0
    nloc = n_nodes // N_CORES
    half = (n_nodes + 1) // 2
    assert half <= 32767 and nloc <= 32767
    nwin = (nloc + WIN - 1) // WIN

    csrc = co[src]            # host-side coord gather (24B/edge stream)
    cdst = co[dst]

    core_of = dst // nloc
    dstl_all = dst - core_of * nloc

    # per-core edge selections, grouped by (window, src-half)
    per_core = []
    counts = np.zeros((N_CORES, nwin, 2), np.int64)
    for c in range(N_CORES):
        sel = np.nonzero(core_of == c)[0]
        w = dstl_all[sel] // WIN
        h = src[sel] // half
        order = np.lexsort((h, w))
        sel = sel[order]
        w, h = w[order], h[order]
        for wi in range(nwin):
            for hi in range(2):
                counts[c, wi, hi] = int(np.sum((w == wi) & (h == hi)))
        per_core.append((sel, w, h))

    # static tile counts per (window, half): max over cores, >=1 tile/window
    tiles_wh = np.zeros((nwin, 2), np.int64)
    for wi in range(nwin):
        for hi in range(2):
            tiles_wh[wi, hi] = -(-counts[:, wi, hi].max() // P)
        if tiles_wh[wi].sum() == 0:
            tiles_wh[wi, 0] = 1
    T = int(tiles_wh.sum())          # total 128-edge tiles per core
    E = T * P                        # padded edge count per core

    # schedule: tile -> (window, half); runs; chunks; windows
    tw = np.zeros(T, np.int64)
    th = np.zeros(T, np.int64)
    t = 0
    for wi in range(nwin):
        for hi in range(2):
            for _ in range(int(tiles_wh[wi, hi])):
                tw[t], th[t] = wi, hi
                t += 1
    runs = []      # (t0, ntiles, half)
    t = 0
    while t < T:
        t0 = t
        while (t < T and th[t] == th[t0] and t - t0 < MAX_RUN_TILES):
            t += 1
        runs.append((t0, t - t0, int(th[t0])))
    chunks = []    # (t0, ntiles)  within runs
    for (t0, nt, _h) in runs:
        s = t0
        while s < t0 + nt:
            n = min(CHUNK_TILES, t0 + nt - s)
            chunks.append((s, n))
            s += n
    windows = []   # (first_tile, ntiles, node_off, width)
    t = 0
    for wi in range(nwin):
        ntl = int(tiles_wh[wi].sum())
        windows.append((t, ntl, wi * WIN, min(WIN, nloc - wi * WIN)))
        t += ntl

    sched = dict(T=T, E=E, runs=runs, chunks=chunks, windows=windows,
                 nloc=nloc, nwin=nwin, half=half, n_nodes=n_nodes)

    # ---- replicated tensors -------------------------------------------------
    nfb = _bf(nf)
    rep = {
        "tabA": nfb[:half],
        "tabB": nfb[half:],
        "We1a": _bf(inputs["We1"][:128]),
        "We1b": _bf(inputs["We1"][128:256]),
        "We1c": _bf(inputs["We1"][256:320]),
        "We2": _bf(inputs["We2"]),
        "Wc1": _bf(inputs["Wc1"]),
        "Wc2": _bf(inputs["Wc2"]),
        "Wn1a": _bf(inputs["Wn1"][:128]),
        "Wn1b": _bf(inputs["Wn1"][128:256]),
        "Wn2": _bf(inputs["Wn2"]),
        "be1": _f32(np.asarray(inputs["be1"]).reshape(128, 1)),
        "be2": _f32(np.asarray(inputs["be2"]).reshape(128, 1)),
        "bc1": _f32(np.asarray(inputs["bc1"]).reshape(64, 1)),
        "bn1": _f32(np.asarray(inputs["bn1"]).reshape(128, 1)),
        "bn2": _f32(np.asarray(inputs["bn2"]).reshape(128, 1)),
    }
    consts = dict(bc2=float(np.asarray(inputs["bc2"]).reshape(-1)[0]))

    # ---- per-core streams ---------------------------------------------------
    def wrap16(idx):
        # dma_gather index layout: element i of a run at [i%16, run_off/16 + i/16],
        # replicated 8x along partitions (one copy per GPSIMD Q7 core).
        out = np.zeros((16, E // 16), np.int16)
        for (t0, nt, _h) in runs:
            s, L = t0 * P, nt * P
            out[:, s // 16:(s + L) // 16] = (
                idx[s:s + L].reshape(L // 16, 16).T)
        return np.ascontiguousarray(np.tile(out, (8, 1)))

    in_maps = []
    for c in range(N_CORES):
        sel, w, h = per_core[c]
        # scatter core-c edges into padded tile layout
        isrc = np.zeros(E, np.int32)
        idst = np.zeros(E, np.int32)
        dstadj = np.full(E, -1.0, np.float32)
        eaT = np.zeros((64, E), ml_dtypes.bfloat16)
        csP = np.zeros((E, 3), np.float32)
        cdP = np.zeros((E, 3), np.float32)
        pos = 0
        ei0 = 0
        for wi in range(nwin):
            for hi in range(2):
                n = int(counts[c, wi, hi])
                cap = int(tiles_wh[wi, hi]) * P
                e = sel[ei0:ei0 + n]
                isrc[pos:pos + n] = src[e] - hi * half
                idst[pos:pos + n] = dstl_all[e]
                dstadj[pos:pos + n] = (dstl_all[e] - wi * WIN).astype(np.float32)
                eaT[:, pos:pos + n] = _bf(ea[e]).T
                csP[pos:pos + n] = csrc[e]
                cdP[pos:pos + n] = cdst[e]
                pos += cap
                ei0 += n
        assert pos == E and ei0 == len(sel)

        nfT_loc = nf[c * nloc:(c + 1) * nloc].T  # [128, nloc]
        m = {
            "idx_src": wrap16(isrc),
            "idx_dst": wrap16(idst),
            "dstadj": np.ascontiguousarray(
                dstadj.reshape(T, P).T),                       # [128, T] f32
            "eaT": np.ascontiguousarray(eaT),                  # [64, E] bf16
            "csP": np.ascontiguousarray(
                csP.reshape(T, P, 3).transpose(1, 0, 2).reshape(P, 3 * T)),
            "cdP": np.ascontiguousarray(
                cdP.reshape(T, P, 3).transpose(1, 0, 2).reshape(P, 3 * T)),
            "tabL": nfb[c * nloc:(c + 1) * nloc],              # [nloc,128] bf16
            "nfT_b": _bf(nfT_loc),                             # [128,nloc]
            "nfT_f": _f32(nfT_loc),
            "co_loc": _f32(co[c * nloc:(c + 1) * nloc]),       # [nloc,3]
        }
        m.update(rep)
        in_maps.append(m)

    return sched, consts, in_maps


# ---------------------------------------------------------------------------
# Device program
# ---------------------------------------------------------------------------

def _build(sched, consts, shapes):
    T, E = sched["T"], sched["E"]
    nloc, half, n_nodes = sched["nloc"], sched["half"], sched["n_nodes"]
    runs, chunks, windows = sched["runs"], sched["chunks"], sched["windows"]

    nc = bacc.Bacc("TRN2", target_bir_lowering=False, debug=False,
                   num_devices=N_CORES)

    def din(name, shape, dt):
        return nc.dram_tensor(name, list(shape), dt, kind="ExternalInput")

    tabA = din("tabA", (half, 128), BF16)
    tabB = din("tabB", (n_nodes - half, 128), BF16)
    tabL = din("tabL", (nloc, 128), BF16)
    idx_src = din("idx_src", (P, E // 16), I16)
    idx_dst = din("idx_dst", (P, E // 16), I16)
    dstadj = din("dstadj", (P, T), F32)
    eaT = din("eaT", (64, E), BF16)
    csP = din("csP", (P, 3 * T), F32)
    cdP = din("cdP", (P, 3 * T), F32)
    nfT_b = din("nfT_b", (P, nloc), BF16)
    nfT_f = din("nfT_f", (P, nloc), F32)
    co_loc = din("co_loc", (nloc, 3), F32)
    wts = {n: din(n, shapes[n], BF16)
           for n in ["We1a", "We1b", "We1c", "We2", "Wc1", "Wc2",
                     "Wn1a", "Wn1b", "Wn2"]}
    bias = {n: din(n, shapes[n], F32)
            for n in ["be1", "be2", "bc1", "bn1", "bn2"]}

    out_nodesT = nc.dram_tensor("out_nodesT", [P, nloc], F32,
                                kind="ExternalOutput")
    out_coords = nc.dram_tensor("out_coords", [nloc, 3], F32,
                                kind="ExternalOutput")

    bc2 = consts["bc2"]

    with tile.TileContext(nc) as tc:
        with (
            tc.tile_pool(name="const", bufs=1) as p_const,
            tc.tile_pool(name="gsrc", bufs=3) as p_gsrc,
            tc.tile_pool(name="gdst", bufs=3) as p_gdst,
            tc.tile_pool(name="ea", bufs=3) as p_ea,
            tc.tile_pool(name="act", bufs=3) as p_act,
            tc.tile_pool(name="srhs", bufs=3) as p_srhs,
            tc.tile_pool(name="coord", bufs=3) as p_co,
            tc.tile_pool(name="small", bufs=3) as p_small,
            tc.tile_pool(name="oh", bufs=4) as p_oh,
            tc.tile_pool(name="wclose", bufs=2) as p_wc,
            tc.tile_pool(name="psA", bufs=2, space="PSUM") as pp_big,
            tc.tile_pool(name="psT", bufs=2, space="PSUM") as pp_tp,
            tc.tile_pool(name="psW", bufs=2, space="PSUM") as pp_win,
            tc.tile_pool(name="psC", bufs=2, space="PSUM") as pp_cw,
        ):
            # ---------------- setup ----------------
            ident_f = p_const.tile([P, P], F32)
            make_identity(nc, ident_f[:])
            ident_b = p_const.tile([P, P], BF16)
            nc.vector.tensor_copy(ident_b[:], ident_f[:])

            iota_i = p_const.tile([P, P], I32)
            nc.gpsimd.iota(iota_i[:], pattern=[[1, P]], base=0,
                           channel_multiplier=0)
            iota_f = p_const.tile([P, P], F32)
            nc.vector.tensor_copy(iota_f[:], iota_i[:])

            sb = {}
            for n, d in list(wts.items()) + list(bias.items()):
                t_ = p_const.tile(list(d.shape), d.dtype, tag=f"sb_{n}",
                                  name=f"sb_{n}")
                nc.sync.dma_start(t_[:], d[:])
                sb[n] = t_
            idxS = p_const.tile([P, E // 16], I16)
            nc.sync.dma_start(idxS[:], idx_src[:])
            idxD = p_const.tile([P, E // 16], I16)
            nc.sync.dma_start(idxD[:], idx_dst[:])
            dadj = p_const.tile([P, T], F32)
            nc.sync.dma_start(dadj[:], dstadj[:])

            # ---- coord diffs + rsqrt(|diff|^2) for ALL edges, once --------
            MUL = mybir.AluOpType.mult
            csA = p_const.tile([P, 3 * T], F32)
            nc.sync.dma_start(csA[:], csP[:])
            cdA = p_const.tile([P, 3 * T], F32)
            nc.sync.dma_start(cdA[:], cdP[:])
            dfA = p_const.tile([P, 3 * T], F32)
            nc.vector.tensor_tensor(out=dfA[:], in0=csA[:], in1=cdA[:],
                                    op=mybir.AluOpType.subtract)
            sqA = p_const.tile([P, 3 * T], F32)
            nc.vector.tensor_tensor(out=sqA[:], in0=dfA[:], in1=dfA[:],
                                    op=MUL)
            xA = p_const.tile([P, T], F32)
            nc.vector.tensor_reduce(
                xA[:], sqA[:].rearrange("p (t c) -> p t c", c=3),
                axis=mybir.AxisListType.X, op=mybir.AluOpType.add)
            nc.vector.tensor_scalar(xA[:], xA[:], 1e-30, None,
                                    op0=mybir.AluOpType.add)
            # rsqrt via exponent bit-hack + 2 Newton steps (all on DVE; the
            # ACT sqrt table lives in a different table set than silu)
            jA = p_const.tile([P, T], I32)
            nc.vector.tensor_scalar(jA[:], xA[:].bitcast(I32), 1, None,
                                    op0=mybir.AluOpType.arith_shift_right)
            magA = p_const.tile([P, T], I32)
            nc.vector.memset(magA[:], 0x5F3759DF)
            nc.vector.tensor_tensor(out=jA[:], in0=magA[:], in1=jA[:],
                                    op=mybir.AluOpType.subtract)
            rinvA = p_const.tile([P, T], F32)
            nc.vector.tensor_copy(rinvA[:], jA[:].bitcast(F32))
            tA = p_const.tile([P, T], F32)
            for _ in range(2):
                nc.vector.tensor_tensor(out=tA[:], in0=xA[:], in1=rinvA[:],
                                        op=MUL)
                nc.vector.tensor_tensor(out=tA[:], in0=tA[:], in1=rinvA[:],
                                        op=MUL)
                nc.vector.tensor_scalar(tA[:], tA[:], -0.5, 1.5,
                                        op0=MUL, op1=mybir.AluOpType.add)
                nc.vector.tensor_tensor(out=rinvA[:], in0=rinvA[:],
                                        in1=tA[:], op=MUL)

            win_ps = [None] * len(windows)
            win_of_tile = {}
            for iw, (wt0, wnt, _no, _wd) in enumerate(windows):
                for tt in range(wt0, wt0 + wnt):
                    win_of_tile[tt] = iw
            last_tile_of_win = {wt0 + wnt - 1: iw
                                for iw, (wt0, wnt, _no, _wd)
                                in enumerate(windows)}
            first_tile_of_win = {wt0: iw
                                 for iw, (wt0, wnt, _no, _wd)
                                 in enumerate(windows)}

            def close_window(iw):
                wt0, wnt, noff, wd = windows[iw]
                ps = win_ps[iw]
                # node aggregation [n, h] -> bf16 -> transpose -> [h, n]
                agT = p_wc.tile([P, P], BF16, tag="agT")
                nc.vector.tensor_copy(agT[:], ps[:, 0:128])
                agP = pp_tp.tile([P, P], BF16, tag="tp")
                nc.tensor.transpose(agP[:], agT[:], ident_b[:])
                ag = p_wc.tile([P, P], BF16, tag="ag")
                nc.vector.tensor_copy(ag[:], agP[:])
                # node MLP
                nfb_w = p_wc.tile([P, wd], BF16, tag="nfbw")
                nc.sync.dma_start(nfb_w[:], nfT_b[:, noff:noff + wd])
                u1 = pp_big.tile([P, wd], F32, tag="big")
                nc.tensor.matmul(u1[:], sb["Wn1a"][:], nfb_w[:],
                                 start=True, stop=False)
                nc.tensor.matmul(u1[:], sb["Wn1b"][:], ag[:, :wd],
                                 start=False, stop=True)
                an = p_wc.tile([P, wd], BF16, tag="an")
                nc.scalar.activation(an[:], u1[:],
                                     mybir.ActivationFunctionType.Silu,
                                     bias=sb["bn1"][:, :1])
                u2 = pp_big.tile([P, wd], F32, tag="big")
                nc.tensor.matmul(u2[:], sb["Wn2"][:], an[:],
                                 start=True, stop=True)
                nff_w = p_wc.tile([P, wd], F32, tag="nffw")
                nc.sync.dma_start(nff_w[:], nfT_f[:, noff:noff + wd])
                o1 = p_wc.tile([P, wd], F32, tag="o1")
                nc.vector.tensor_scalar(o1[:], u2[:], sb["bn2"][:, :1], None,
                                        op0=mybir.AluOpType.add)
                nc.vector.tensor_add(o1[:], o1[:], nff_w[:])
                nc.sync.dma_start(out_nodesT[:, noff:noff + wd], o1[:])
                # coords
                cl = p_wc.tile([wd, 3], F32, tag="cl")
                nc.sync.dma_start(cl[:], co_loc[noff:noff + wd, :])
                oc = p_wc.tile([wd, 3], F32, tag="oc")
                nc.vector.tensor_add(oc[:], ps[:wd, 128:131], cl[:])
                nc.sync.dma_start(out_coords[noff:noff + wd, :], oc[:])
                win_ps[iw] = None

            # ---------------- main loop ----------------
            run_tiles = {}
            ci = 0
            for ri, (rt0, rnt, rhalf) in enumerate(runs):
                L = rnt * P
                nfsT = p_gsrc.tile([P, L], BF16, tag="gs")
                nfdT = p_gdst.tile([P, L], BF16, tag="gd")
                tab = tabA if rhalf == 0 else tabB
                nc.gpsimd.dma_gather(
                    out_ap=nfsT[:].rearrange("p (o n) -> p o n", o=1),
                    in_ap=tab[:],
                    idxs_ap=idxS[:, (rt0 * P) // 16:(rt0 * P + L) // 16],
                    num_idxs=L, num_idxs_reg=L, elem_size=128,
                    transpose=True)
                nc.gpsimd.dma_gather(
                    out_ap=nfdT[:].rearrange("p (o n) -> p o n", o=1),
                    in_ap=tabL[:],
                    idxs_ap=idxD[:, (rt0 * P) // 16:(rt0 * P + L) // 16],
                    num_idxs=L, num_idxs_reg=L, elem_size=128,
                    transpose=True)

                while ci < len(chunks) and chunks[ci][0] < rt0 + rnt:
                    t0, nt = chunks[ci]
                    ci += 1
                    W = nt * P
                    off = (t0 - rt0) * P

                    ea_t = p_ea.tile([64, W], BF16, tag="ea")
                    nc.sync.dma_start(ea_t[:], eaT[:, t0 * P:t0 * P + W])

                    h1 = pp_big.tile([P, W], F32, tag="big")
                    nc.tensor.matmul(h1[:], sb["We1a"][:],
                                     nfsT[:, off:off + W],
                                     start=True, stop=False)
                    nc.tensor.matmul(h1[:], sb["We1b"][:],
                                     nfdT[:, off:off + W],
                                     start=False, stop=False)
                    nc.tensor.matmul(h1[:], sb["We1c"][:], ea_t[:],
                                     start=False, stop=True)
                    a1 = p_act.tile([P, W], BF16, tag="a1")
                    nc.scalar.activation(a1[:], h1[:],
                                         mybir.ActivationFunctionType.Silu,
                                         bias=sb["be1"][:, :1])
                    mp = pp_big.tile([P, W], F32, tag="big")
                    nc.tensor.matmul(mp[:], sb["We2"][:], a1[:],
                                     start=True, stop=True)
                    msg = p_act.tile([P, W], BF16, tag="msg")
                    nc.scalar.activation(msg[:], mp[:],
                                         mybir.ActivationFunctionType.Silu,
                                         bias=sb["be2"][:, :1])
                    hp = pp_big.tile([64, W], F32, tag="big")
                    nc.tensor.matmul(hp[:], sb["Wc1"][:], msg[:],
                                     start=True, stop=True)
                    hc = p_act.tile([64, W], BF16, tag="hc")
                    nc.scalar.activation(hc[:], hp[:],
                                         mybir.ActivationFunctionType.Silu,
                                         bias=sb["bc1"][:, :1])

                    # per-edge coord weight cw = hc.T @ Wc2  -> [e, 1] slices
                    cwp = pp_cw.tile([P, nt], F32, tag="cw")
                    for k in range(nt):
                        nc.tensor.matmul(cwp[:, k:k + 1],
                                         hc[:, k * P:(k + 1) * P],
                                         sb["Wc2"][:, :1],
                                         start=True, stop=True,
                                         skip_group_check=True)

                    # coord weight: wf = (cw + bc2) * rsqrt(|diff|^2)
                    cwb = p_small.tile([P, nt], F32, tag="cwb")
                    nc.vector.tensor_scalar(cwb[:], cwp[:, :nt], bc2, None,
                                            op0=mybir.AluOpType.add)
                    wf = p_small.tile([P, nt], F32, tag="wf")
                    nc.vector.tensor_tensor(
                        out=wf[:], in0=cwb[:],
                        in1=rinvA[:, t0:t0 + nt],
                        op=mybir.AluOpType.mult)

                    # scatter rhs = [msgT | wdiff] per tile
                    srhs = p_srhs.tile([P, nt, 132], BF16, tag="srhs")
                    for k in range(nt):
                        tp = pp_tp.tile([P, P], BF16, tag="tp")
                        nc.tensor.transpose(tp[:], msg[:, k * P:(k + 1) * P],
                                            ident_b[:])
                        nc.vector.tensor_copy(srhs[:, k, 0:128], tp[:])
                        nc.vector.tensor_scalar(
                            srhs[:, k, 128:131],
                            dfA[:, 3 * (t0 + k):3 * (t0 + k) + 3],
                            wf[:, k:k + 1], None, op0=mybir.AluOpType.mult)

                    # one-hot scatter into window PSUM
                    for k in range(nt):
                        tt = t0 + k
                        iw = win_of_tile[tt]
                        if tt in first_tile_of_win:
                            wtile = pp_win.tile([P, 132], F32, tag="win",
                                                name=f"win{iw}")
                            win_ps[iw] = wtile
                        oh = p_oh.tile([P, P], BF16, tag="oh")
                        nc.vector.tensor_scalar(
                            oh[:], iota_f[:], dadj[:, tt:tt + 1], None,
                            op0=mybir.AluOpType.is_equal)
                        nc.tensor.matmul(
                            win_ps[iw][:, 0:131], oh[:], srhs[:, k, 0:131],
                            start=(tt in first_tile_of_win),
                            stop=(tt in last_tile_of_win),
                            skip_group_check=True)
                        if tt in last_tile_of_win:
                            close_window(iw)

    nc.compile()
    return nc


# ---------------------------------------------------------------------------
# Entry point
# ---------------------------------------------------------------------------

LAST_NC = None
LAST_IN_MAPS = None


def kernel(**inputs):
    global LAST_RESULTS, LAST_NC, LAST_IN_MAPS
    sched, consts, in_maps = _prepare(inputs)
    shapes = {n: in_maps[0][n].shape
              for n in ["We1a", "We1b", "We1c", "We2", "Wc1", "Wc2",
                        "Wn1a", "Wn1b", "Wn2",
                        "be1", "be2", "bc1", "bn1", "bn2"]}
    nc = _build(sched, consts, shapes)
    LAST_NC, LAST_IN_MAPS = nc, in_maps

    trace = bool(int(os.environ.get("KERNEL_TRACE", "0")))
    res = run_bass_kernel_spmd(nc, in_maps, list(range(N_CORES)),
                               trace=trace)
    LAST_RESULTS = res

    nodes = np.concatenate(
        [np.ascontiguousarray(res.results[c]["out_nodesT"].T)
         for c in range(N_CORES)], axis=0)
    coords = np.concatenate(
        [res.results[c]["out_coords"] for c in range(N_CORES)], axis=0)
    return nodes, coords
